# revision 28
# baseline (speedup 1.0000x reference)
"""Trainium2 Bass kernel for nn_ConditionedCategorical (segment_reduce).

Computes, for x_labels [N] (values in [0,16)), y_labels [N] (values in
[0,32)), posterior_estimate [N, 16] fp32:

    numerator[k, y, :] = eps + sum_{n: x_n=k, y_n=y} posterior[n, :]
    out = numerator / numerator.sum(axis=1, keepdims=True)      # [16, 32, 16]

Strategy ("segment-parallel", 8 NeuronCores, no collective):
  - Core c owns segments s = x*32+y in [64c, 64c+64) outright (x in
    [2c, 2c+2)): the host sorts rows by s and ships each core ALL rows of
    its 64 segments, so every segment is a ~64-tile mono-segment run and
    the Y-normalization is core-local.  The host pads local segment i to
    the max row count across cores (~2.7%) so all cores share ONE SPMD
    instruction stream, then concatenates the per-core [2,32,16] outputs.
  - Posterior is pre-cast to fp8e4m3 (16 B/row, the only device input).
  - PSUM acc [16, 4096]: row = i&15, free = ((i>>4)*16 + lane)*16 + c.
    Mono-segment runs go through plain matmuls, one tile per lane, up to
    16 lanes (N=256) per instruction, with a constant ones-at-row-j lhsT
    sliced from a tiny two-spike table; a tile spanning segments i,i+1
    uses a staircase [p<q | p>=q] lhsT pair at rows j,j+1 (one matmul,
    lane 0).  Weights are tiny (M=16) so LDWEIGHTS stays off the
    critical path.  (DoubleRow measured ~9us SLOWER here despite half
    the stream cycles - its LDWEIGHTS/overhead dominates at M=16.)
  - Per-group lane-reduces on DVE overlap the tail of the main loop;
    epilogue is a small rearrange + eps + normalize, all core-local.
"""

import numpy as np
import ml_dtypes

import concourse.bass as bass
import concourse.tile as tile
from concourse import bacc, mybir
from concourse.bass_utils import run_bass_kernel_spmd

K, Y, C = 16, 32, 16
NSEG = K * Y         # 512 global segments
LSEG = NSEG // 8     # 64 local segments per core
EPS = 1e-8
NCORES = 8
P = 128
ST = 512             # tiles per DMA chunk (8 KiB per partition line)
G = 16               # PSUM lanes per (row, group)

f32 = mybir.dt.float32
f16 = mybir.dt.float16
i16 = mybir.dt.int16
f8e4 = mybir.dt.float8e4
f8e5 = mybir.dt.float8e5
DR = mybir.MatmulPerfMode.DoubleRow

# last BassKernelResults (for test harness inspection)
last_results = None


def _plan_ops(meta, st, use_dr=True):
    """Turn per-tile metadata into per-chunk matmul ops.
    meta = (nt, segA, q): segA[t] = local segment of tile t's first row;
    q[t] == 128 marks a mono-segment tile, else rows [0,q) belong to
    segA[t] and rows [q,128) to segA[t]+1.
    Returns chunks = [(c0, csz, [op...])]; op kinds:
      ("dr",   rel, i, nlane)  DoubleRow, tiles rel..rel+2*nlane-1 of seg i
      ("mm",   rel, i, L)      plain matmul, tiles rel..rel+L-1, one/lane
      ("mono", rel, i, 0)      single leftover tile of seg i
      ("bnd",  rel, i, q)      boundary tile split at q between i, i+1
    """
    nt, segA, qs = meta
    chunks = []
    for c0 in range(0, nt, st):
        csz = min(st, nt - c0)
        ops = []
        i = c0
        while i < c0 + csz:
            s, q = segA[i], qs[i]
            if q != 128:
                assert 0 < q < 128 and (s & 15) != 15
                ops.append(("bnd", i - c0, s, q))
                i += 1
                continue
            r = i
            while r < c0 + csz and qs[r] == 128 and segA[r] == s:
                r += 1
            run = r - i
            if use_dr:
                while run >= 2:
                    nl = min(G, run // 2)
                    ops.append(("dr", i - c0, s, nl))
                    i += 2 * nl
                    run -= 2 * nl
                if run == 1:
                    ops.append(("mono", i - c0, s, 0))
                    i += 1
            else:
                while run > 0:
                    L = min(G, run)
                    ops.append(("mm", i - c0, s, L))
                    i += L
                    run -= L
        chunks.append((c0, csz, ops))
    return chunks


def build_nc(meta, st=ST, repeat=1, single_core=False, no_mm=False,
             empty=False, use_dr=False, bufs=8):
    """repeat>1 wraps the main loop in a hardware For_i (PSUM re-zeroed
    each pass; used for steady-state timing). single_core builds a
    1-device module (core 0's data only). use_dr=False replaces DoubleRow
    pair-lane matmuls with plain one-tile-per-lane matmuls (cheap
    LDWEIGHTS, 2x stream cycles). Timing-isolation variants (wrong
    results): no_mm drops the matmuls+reduces (DMA-only), empty drops the
    whole chunk loop (For_i overhead only; WEDGES THE DEVICE, avoid)."""
    nt = meta[0]
    chunks = _plan_ops(meta, st, use_dr=use_dr)

    ndev = 1 if single_core else NCORES
    nc = bacc.Bacc("TRN2", target_bir_lowering=False, debug=False,
                   num_devices=ndev)

    post = nc.declare_dram_parameter("post", [P, nt * C], f8e4, isOutput=False)
    out = nc.declare_dram_parameter("out", [2, Y, C], f32, isOutput=True)

    with tile.TileContext(nc) as tc:
        with (
            tc.tile_pool(name="setup", bufs=1) as setup,
            tc.tile_pool(name="postp", bufs=bufs) as post_pool,
            tc.tile_pool(name="acc", bufs=1, space="PSUM") as acc_pool,
            tc.tile_pool(name="epi", bufs=1) as epi,
            tc.tile_pool(name="dram", bufs=1, space="DRAM") as dram,
        ):
            # --- setup: constant lhsT tables (fp16 built, fp8e5m2 viewed:
            # the high byte of fp16 1.0/0.0 is exactly e5m2 1.0/0.0) -------
            # U: two-spike table for mono runs. U[:,15]=U[:,31]=1, else 0.
            #   DR lhsT(j)    = U8[:, 15-j : 47-j]  as [p, two=2, m=16]
            #   single lhsT(j)= U8[:, 15-j : 31-j]  (ones at row j)
            U16 = setup.tile([P, 47], f16)
            nc.vector.memset(U16[:], 0.0)
            nc.vector.memset(U16[:, 15:16], 1.0)
            nc.vector.memset(U16[:, 31:32], 1.0)
            U8 = U16[:].bitcast(f8e5)[:, 1::2]
            # T: staircase table for boundary tiles, 32-col block per split
            # q: col q*32+15 = (p<q), col q*32+16 = (p>=q), zeros elsewhere.
            #   bnd lhsT(q, j) = T8[:, q*32+15-j : q*32+31-j]
            iota_p = setup.tile([P, 1], i16)
            nc.gpsimd.iota(iota_p[:], pattern=[[0, 1]], base=0,
                           channel_multiplier=1)
            pf = setup.tile([P, 1], f16)
            nc.vector.tensor_copy(pf[:], iota_p[:])
            iota_q = setup.tile([P, 129], i16)
            nc.gpsimd.iota(iota_q[:], pattern=[[1, 129]], base=0,
                           channel_multiplier=0)
            qf = setup.tile([P, 129], f16)
            nc.vector.tensor_copy(qf[:], iota_q[:])
            T16 = setup.tile([P, 129 * 32], f16)
            nc.vector.memset(T16[:], 0.0)
            nc.vector.tensor_tensor(
                T16[:, 15::32], pf[:].broadcast_to((P, 129)), qf[:],
                op=mybir.AluOpType.is_lt)
            nc.vector.tensor_tensor(
                T16[:, 16::32], pf[:].broadcast_to((P, 129)), qf[:],
                op=mybir.AluOpType.is_ge)
            T8 = T16[:].bitcast(f8e5)[:, 1::2]

            # acc[j, ((grp*G)+lane)*16+c]: local seg i = grp*16+j
            acc = acc_pool.tile([16, 4 * G * C], f32, name="acc", tag="acc")
            red = epi.tile([16, 4 * C], f32)



            def main_pass():
                # zero the accumulator, split across DVE + Act so it
                # overlaps the first DMA chunk
                half = 2 * G * C
                nc.vector.memset(acc[:, 0:half], 0.0)
                nc.scalar.memzero(acc[:, half:2 * half])
                if empty:
                    return
                for ci, (c0, csz, ops) in enumerate(chunks):
                    pf8 = post_pool.tile([P, csz * C], f8e4, name="pf8",
                                         tag="pf8")
                    nc.sync.dma_start(pf8[:], post[:, c0 * C:(c0 + csz) * C])
                    if no_mm:
                        continue
                    for oi, (kind, rt, s, arg) in enumerate(ops):
                        grp, j = s >> 4, s & 15
                        gbase = grp * G * C
                        if kind == "dr":
                            # pair tile blocks [rt, rt+nl) x [rt+nl, rt+2nl)
                            # (same segment, so any pairing sums correctly);
                            # keeps the rhs a clean contiguous 3D DR AP
                            nl = arg
                            nc.tensor.matmul(
                                acc[:, gbase:gbase + nl * C],
                                lhsT=U8[:, 15 - j:47 - j].rearrange(
                                    "p (two m) -> p two m", two=2),
                                rhs=pf8[:, rt * C:(rt + 2 * nl) * C]
                                    .rearrange("p (two gn) -> p two gn",
                                               two=2),
                                start=False, stop=False, perf_mode=DR,
                                skip_group_check=True,
                            )
                        elif kind == "mm":
                            L = arg
                            nc.tensor.matmul(
                                acc[:, gbase:gbase + L * C],
                                lhsT=U8[:, 15 - j:31 - j],
                                rhs=pf8[:, rt * C:(rt + L) * C],
                                start=False, stop=False,
                                skip_group_check=True,
                            )
                        elif kind == "mono":
                            nc.tensor.matmul(
                                acc[:, gbase:gbase + C],
                                lhsT=U8[:, 15 - j:31 - j],
                                rhs=pf8[:, rt * C:(rt + 1) * C],
                                start=False, stop=False,
                                skip_group_check=True,
                            )
                        else:   # bnd: rows [0,q)->seg s, [q,128)->seg s+1
                            q = arg
                            nc.tensor.matmul(
                                acc[:, gbase:gbase + C],
                                lhsT=T8[:, q * 32 + 15 - j:q * 32 + 31 - j],
                                rhs=pf8[:, rt * C:(rt + 1) * C],
                                start=False, stop=False,
                                skip_group_check=True,
                            )
            if repeat > 1:
                with tc.For_i(0, repeat):
                    main_pass()
            else:
                main_pass()
            if not (no_mm or empty):
                # lane-reduce: acc [16, (grp, lane, c)] -> red [16, (grp, c)]
                # (after the loop: one-time work, reads the last pass's acc)
                for grp in range(4):
                    gb = grp * G * C
                    nc.vector.tensor_reduce(
                        red[:, grp * C:(grp + 1) * C],
                        acc[:, gb:gb + G * C].rearrange(
                            "p (g c) -> p c g", c=C),
                        axis=mybir.AxisListType.X,
                        op=mybir.AluOpType.add,
                    )
            if no_mm or empty:
                # timing-only variants never write red; keep the epilogue
                # reads legal
                nc.vector.memset(red[:], 0.0)

            # --- epilogue (core-local): rearrange, eps, normalize ----------
            # red[y&15, (xl*2 + (y>>4))*16 + c] -> tmp[xl, y, c]
            tmp = dram.tile([2, Y, C], f32)
            for xl in range(2):
                for yhi in range(2):
                    grp = xl * 2 + yhi
                    nc.sync.dma_start(
                        tmp[xl, yhi * 16:(yhi + 1) * 16, :],
                        red[:, grp * C:(grp + 1) * C],
                    )
            num = epi.tile([2, Y * C], f32)
            nc.sync.dma_start(num[:], tmp[:].rearrange("k y c -> k (y c)"))
            nc.vector.tensor_scalar(
                num[:], num[:], EPS, None, mybir.AluOpType.add)
            den = epi.tile([2, C], f32)
            nc.vector.tensor_reduce(
                den[:],
                num[:].rearrange("k (y c) -> k c y", c=C),
                axis=mybir.AxisListType.X,
                op=mybir.AluOpType.add,
            )
            rec = epi.tile([2, C], f32)
            nc.vector.reciprocal(rec[:], den[:])
            norm = epi.tile([2, Y * C], f32)
            nc.vector.tensor_tensor(
                norm[:].rearrange("k (y c) -> k y c", c=C),
                num[:].rearrange("k (y c) -> k y c", c=C),
                rec[:].unsqueeze(1).broadcast_to((2, Y, C)),
                op=mybir.AluOpType.mult,
            )
            nc.sync.dma_start(
                out[:].rearrange("k y c -> k (y c)"), norm[:])

    nc.compile()
    return nc


_nc_cache = {}


def _get_nc(meta, st, repeat=1, single_core=False):
    key = (meta, st, repeat, single_core)
    if key not in _nc_cache:
        _nc_cache[key] = build_nc(meta, st, repeat=repeat,
                                  single_core=single_core)
    return _nc_cache[key]


def prep_in_maps(inputs):
    """Host prep: sort rows by segment; core c gets all rows of segments
    [64c, 64c+64); shared slot layout padded to the cross-core max per
    local segment; cast to fp8; partition-major. Returns (in_maps, meta,
    st)."""
    x = np.asarray(inputs["x_labels"]).astype(np.int64)
    y = np.asarray(inputs["y_labels"]).astype(np.int64)
    post = np.asarray(inputs["posterior_estimate"], dtype=np.float32)
    n = x.shape[0]
    s = (x * Y + y).astype(np.int32)
    post8 = post.astype(ml_dtypes.float8_e4m3)

    T = np.bincount(s, minlength=NSEG)
    order = np.argsort(s, kind="stable")
    s_sorted = s[order]
    seg_start = np.concatenate([[0], np.cumsum(T)])
    rk = np.arange(n) - seg_start[s_sorted]

    # local segment i spans [off[i], off[i+1]); 128-aligned after every
    # 16th segment so no boundary tile pairs PSUM row 15 with row 16.
    Lmax = T.reshape(NCORES, LSEG).max(axis=0)
    off = np.zeros(LSEG + 1, np.int64)
    cum = 0
    for i in range(LSEG):
        off[i] = cum
        cum += int(Lmax[i])
        if (i & 15) == 15:
            cum = (cum + 127) // 128 * 128
    off[LSEG] = cum
    assert cum % 128 == 0
    nt = int(cum) // 128

    lo = np.arange(nt, dtype=np.int64) * 128
    segA = np.searchsorted(off, lo, side="right") - 1
    segB = np.searchsorted(off, lo + 127, side="right") - 1
    assert np.all(segB - segA <= 1), "tile spans 3+ segments"
    q = np.where(segA == segB, 128, off[segB] - lo)
    meta = (nt, tuple(int(v) for v in segA), tuple(int(v) for v in q))

    core_of = s_sorted >> 6         # global seg // 64
    dst_all = off[s_sorted & 63] + rk
    in_maps = []
    for c in range(NCORES):
        sel = core_of == c
        pad = np.zeros((nt * P, C), ml_dtypes.float8_e4m3)
        pad[dst_all[sel]] = post8[order[sel]]
        in_maps.append({
            "post": np.ascontiguousarray(
                pad.reshape(nt, P, C).transpose(1, 0, 2).reshape(P, nt * C)),
        })
    return in_maps, meta, ST


def kernel(x_labels, y_labels, posterior_estimate, _trace=False,
           _tmpdir=None):
    global last_results
    in_maps, meta, st = prep_in_maps(dict(
        x_labels=x_labels, y_labels=y_labels,
        posterior_estimate=posterior_estimate,
    ))
    nc = _get_nc(meta, st)
    kwargs = {}
    if _trace:
        kwargs.update(trace=True, tmpdir=_tmpdir)
    res = run_bass_kernel_spmd(nc, in_maps, list(range(NCORES)), **kwargs)
    last_results = res
    return np.concatenate([res.results[c]["out"] for c in range(NCORES)],
                          axis=0)


# revision 34
# speedup vs baseline: 1.1051x; 1.1051x over previous
"""Trainium2 Bass kernel for nn_ConditionedCategorical (segment_reduce).

Computes, for x_labels [N] (values in [0,16)), y_labels [N] (values in
[0,32)), posterior_estimate [N, 16] fp32:

    numerator[k, y, :] = eps + sum_{n: x_n=k, y_n=y} posterior[n, :]
    out = numerator / numerator.sum(axis=1, keepdims=True)      # [16, 32, 16]

Strategy ("segment-parallel", 8 NeuronCores, no collective):
  - Core c owns segments s = x*32+y in [64c, 64c+64) outright (x in
    [2c, 2c+2)): the host sorts rows by s and ships each core ALL rows of
    its 64 segments, so every segment is a ~64-tile mono-segment run and
    the Y-normalization is core-local.  The host pads local segment i to
    the max row count across cores (~2.7%) so all cores share ONE SPMD
    instruction stream, then concatenates the per-core [2,32,16] outputs.
  - Posterior is pre-cast to fp8e4m3 (16 B/row, the only device input).
  - PSUM acc [16, 4096]: row = i&15, free = ((i>>4)*16 + lane)*16 + c.
    Mono-segment runs go through plain matmuls, one tile per lane, up to
    16 lanes (N=256) per instruction, with a constant ones-at-row-j lhsT
    sliced from a tiny two-spike table; a tile spanning segments i,i+1
    uses a staircase [p<q | p>=q] lhsT pair at rows j,j+1 (one matmul,
    lane 0).  Weights are tiny (M=16) so LDWEIGHTS stays off the
    critical path.  (DoubleRow measured ~9us SLOWER here despite half
    the stream cycles - its LDWEIGHTS/overhead dominates at M=16.)
  - Per-group lane-reduces on DVE overlap the tail of the main loop;
    epilogue is a small rearrange + eps + normalize, all core-local.
"""

import numpy as np
import ml_dtypes

import concourse.bass as bass
import concourse.tile as tile
from concourse import bacc, mybir
from concourse.bass_utils import run_bass_kernel_spmd

K, Y, C = 16, 32, 16
NSEG = K * Y         # 512 global segments
LSEG = NSEG // 8     # 64 local segments per core
EPS = 1e-8
NCORES = 8
P = 128
ST = 256             # tiles per DMA chunk (4 KiB per partition line)
G = 16               # PSUM lanes per (row, group)

f32 = mybir.dt.float32
f16 = mybir.dt.float16
i16 = mybir.dt.int16
f8e4 = mybir.dt.float8e4
f8e5 = mybir.dt.float8e5
DR = mybir.MatmulPerfMode.DoubleRow

# last BassKernelResults (for test harness inspection)
last_results = None


def _plan_ops(meta, st, use_dr=True):
    """Turn per-tile metadata into per-chunk matmul ops.
    meta = (nt, segA, q): segA[t] = local segment of tile t's first row;
    q[t] == 128 marks a mono-segment tile, else rows [0,q) belong to
    segA[t] and rows [q,128) to segA[t]+1.
    Returns chunks = [(c0, csz, [op...])]; op kinds:
      ("dr",   rel, i, nlane)  DoubleRow, tiles rel..rel+2*nlane-1 of seg i
      ("mm",   rel, i, L)      plain matmul, tiles rel..rel+L-1, one/lane
      ("mono", rel, i, 0)      single leftover tile of seg i
      ("bnd",  rel, i, q)      boundary tile split at q between i, i+1
    """
    nt, segA, qs = meta
    chunks = []
    for c0 in range(0, nt, st):
        csz = min(st, nt - c0)
        ops = []
        i = c0
        while i < c0 + csz:
            s, q = segA[i], qs[i]
            if q != 128:
                assert 0 < q < 128 and (s & 15) != 15
                ops.append(("bnd", i - c0, s, q))
                i += 1
                continue
            r = i
            while r < c0 + csz and qs[r] == 128 and segA[r] == s:
                r += 1
            run = r - i
            if use_dr:
                while run >= 2:
                    nl = min(G, run // 2)
                    ops.append(("dr", i - c0, s, nl))
                    i += 2 * nl
                    run -= 2 * nl
                if run == 1:
                    ops.append(("mono", i - c0, s, 0))
                    i += 1
            else:
                while run > 0:
                    L = min(G, run)
                    ops.append(("mm", i - c0, s, L))
                    i += L
                    run -= L
        chunks.append((c0, csz, ops))
    return chunks


def build_nc(meta, st=ST, repeat=1, single_core=False, no_mm=False,
             empty=False, use_dr=False, use_drswi=False, bufs=8):
    """repeat>1 wraps the main loop in a hardware For_i (PSUM re-zeroed
    each pass; used for steady-state timing). single_core builds a
    1-device module (core 0's data only). use_dr=False replaces DoubleRow
    pair-lane matmuls with plain one-tile-per-lane matmuls (cheap
    LDWEIGHTS, 2x stream cycles). Timing-isolation variants (wrong
    results): no_mm drops the matmuls+reduces (DMA-only), empty drops the
    whole chunk loop (For_i overhead only; WEDGES THE DEVICE, avoid)."""
    nt = meta[0]
    chunks = _plan_ops(meta, st, use_dr=use_dr or use_drswi)

    ndev = 1 if single_core else NCORES
    nc = bacc.Bacc("TRN2", target_bir_lowering=False, debug=False,
                   num_devices=ndev)

    post = nc.declare_dram_parameter("post", [P, nt * C], f8e4, isOutput=False)
    out = nc.declare_dram_parameter("out", [2, Y, C], f32, isOutput=True)

    with tile.TileContext(nc) as tc:
        with (
            tc.tile_pool(name="setup", bufs=1) as setup,
            tc.tile_pool(name="postp", bufs=bufs) as post_pool,
            tc.tile_pool(name="acc", bufs=1, space="PSUM") as acc_pool,
            tc.tile_pool(name="epi", bufs=1) as epi,
            tc.tile_pool(name="dram", bufs=1, space="DRAM") as dram,
        ):
            # --- setup: constant lhsT tables (fp16 built, fp8e5m2 viewed:
            # the high byte of fp16 1.0/0.0 is exactly e5m2 1.0/0.0) -------
            # U: two-spike table for mono runs. U[:,15]=U[:,31]=1, else 0.
            #   DR lhsT(j)    = U8[:, 15-j : 47-j]  as [p, two=2, m=16]
            #   single lhsT(j)= U8[:, 15-j : 31-j]  (ones at row j)
            U16 = setup.tile([P, 47], f16)
            nc.vector.memset(U16[:], 0.0)
            nc.vector.memset(U16[:, 15:16], 1.0)
            nc.vector.memset(U16[:, 31:32], 1.0)
            U8 = U16[:].bitcast(f8e5)[:, 1::2]
            if use_drswi:
                # V: DoubleRowSwInterleave variant of U — HW deinterleaves
                # even/odd columns into weight A/B and reverses: window
                # [2j, 2j+32) with spikes at global cols 30,31 puts 1.0 at
                # logical row j of both A and B.
                V16 = setup.tile([P, 62], f16)
                nc.vector.memset(V16[:], 0.0)
                nc.vector.memset(V16[:, 30:32], 1.0)
                V8 = V16[:].bitcast(f8e5)[:, 1::2]
            # T: staircase table for boundary tiles, 32-col block per split
            # q: col q*32+15 = (p<q), col q*32+16 = (p>=q), zeros elsewhere.
            #   bnd lhsT(q, j) = T8[:, q*32+15-j : q*32+31-j]
            iota_p = setup.tile([P, 1], i16)
            nc.gpsimd.iota(iota_p[:], pattern=[[0, 1]], base=0,
                           channel_multiplier=1)
            pf = setup.tile([P, 1], f16)
            nc.vector.tensor_copy(pf[:], iota_p[:])
            iota_q = setup.tile([P, 129], i16)
            nc.gpsimd.iota(iota_q[:], pattern=[[1, 129]], base=0,
                           channel_multiplier=0)
            qf = setup.tile([P, 129], f16)
            nc.vector.tensor_copy(qf[:], iota_q[:])
            T16 = setup.tile([P, 129 * 32], f16)
            nc.vector.memset(T16[:], 0.0)
            nc.vector.tensor_tensor(
                T16[:, 15::32], pf[:].broadcast_to((P, 129)), qf[:],
                op=mybir.AluOpType.is_lt)
            nc.vector.tensor_tensor(
                T16[:, 16::32], pf[:].broadcast_to((P, 129)), qf[:],
                op=mybir.AluOpType.is_ge)
            T8 = T16[:].bitcast(f8e5)[:, 1::2]

            # acc[j, ((grp*G)+lane)*16+c]: local seg i = grp*16+j
            acc = acc_pool.tile([16, 4 * G * C], f32, name="acc", tag="acc")
            red = epi.tile([16, 4 * C], f32)



            def main_pass():
                # zero the accumulator, split across DVE + Act so it
                # overlaps the first DMA chunk
                half = 2 * G * C
                nc.vector.memset(acc[:, 0:half], 0.0)
                nc.scalar.memzero(acc[:, half:2 * half])
                if empty:
                    return
                for ci, (c0, csz, ops) in enumerate(chunks):
                    pf8 = post_pool.tile([P, csz * C], f8e4, name="pf8",
                                         tag="pf8")
                    nc.sync.dma_start(pf8[:], post[:, c0 * C:(c0 + csz) * C])
                    if no_mm:
                        continue
                    for oi, (kind, rt, s, arg) in enumerate(ops):
                        grp, j = s >> 4, s & 15
                        gbase = grp * G * C
                        if kind == "dr":
                            # pair tile blocks [rt, rt+nl) x [rt+nl, rt+2nl)
                            # (same segment, so any pairing sums correctly);
                            # keeps the rhs a clean contiguous 3D DR AP
                            nl = arg
                            if use_drswi:
                                lhsT = V8[:, 2 * j:2 * j + 32]
                                pm = mybir.MatmulPerfMode.DoubleRowSwInterleave
                            else:
                                lhsT = U8[:, 15 - j:47 - j].rearrange(
                                    "p (two m) -> p two m", two=2)
                                pm = DR
                            nc.tensor.matmul(
                                acc[:, gbase:gbase + nl * C],
                                lhsT=lhsT,
                                rhs=pf8[:, rt * C:(rt + 2 * nl) * C]
                                    .rearrange("p (two gn) -> p two gn",
                                               two=2),
                                start=False, stop=False, perf_mode=pm,
                                skip_group_check=True,
                            )
                        elif kind == "mm":
                            L = arg
                            nc.tensor.matmul(
                                acc[:, gbase:gbase + L * C],
                                lhsT=U8[:, 15 - j:31 - j],
                                rhs=pf8[:, rt * C:(rt + L) * C],
                                start=False, stop=False,
                                skip_group_check=True,
                            )
                        elif kind == "mono":
                            nc.tensor.matmul(
                                acc[:, gbase:gbase + C],
                                lhsT=U8[:, 15 - j:31 - j],
                                rhs=pf8[:, rt * C:(rt + 1) * C],
                                start=False, stop=False,
                                skip_group_check=True,
                            )
                        else:   # bnd: rows [0,q)->seg s, [q,128)->seg s+1
                            q = arg
                            nc.tensor.matmul(
                                acc[:, gbase:gbase + C],
                                lhsT=T8[:, q * 32 + 15 - j:q * 32 + 31 - j],
                                rhs=pf8[:, rt * C:(rt + 1) * C],
                                start=False, stop=False,
                                skip_group_check=True,
                            )
            if repeat > 1:
                with tc.For_i(0, repeat):
                    main_pass()
            else:
                main_pass()
            if not (no_mm or empty):
                # lane-reduce: acc [16, (grp, lane, c)] -> red [16, (grp, c)]
                # (after the loop: one-time work, reads the last pass's acc)
                for grp in range(4):
                    gb = grp * G * C
                    nc.vector.tensor_reduce(
                        red[:, grp * C:(grp + 1) * C],
                        acc[:, gb:gb + G * C].rearrange(
                            "p (g c) -> p c g", c=C),
                        axis=mybir.AxisListType.X,
                        op=mybir.AluOpType.add,
                    )
            if no_mm or empty:
                # timing-only variants never write red; keep the epilogue
                # reads legal
                nc.vector.memset(red[:], 0.0)

            # --- epilogue (core-local): rearrange, eps, normalize ----------
            # red[y&15, (xl*2 + (y>>4))*16 + c] -> tmp[xl, y, c]
            tmp = dram.tile([2, Y, C], f32)
            for xl in range(2):
                for yhi in range(2):
                    grp = xl * 2 + yhi
                    nc.sync.dma_start(
                        tmp[xl, yhi * 16:(yhi + 1) * 16, :],
                        red[:, grp * C:(grp + 1) * C],
                    )
            num = epi.tile([2, Y * C], f32)
            nc.sync.dma_start(num[:], tmp[:].rearrange("k y c -> k (y c)"))
            nc.vector.tensor_scalar(
                num[:], num[:], EPS, None, mybir.AluOpType.add)
            den = epi.tile([2, C], f32)
            nc.vector.tensor_reduce(
                den[:],
                num[:].rearrange("k (y c) -> k c y", c=C),
                axis=mybir.AxisListType.X,
                op=mybir.AluOpType.add,
            )
            rec = epi.tile([2, C], f32)
            nc.vector.reciprocal(rec[:], den[:])
            norm = epi.tile([2, Y * C], f32)
            nc.vector.tensor_tensor(
                norm[:].rearrange("k (y c) -> k y c", c=C),
                num[:].rearrange("k (y c) -> k y c", c=C),
                rec[:].unsqueeze(1).broadcast_to((2, Y, C)),
                op=mybir.AluOpType.mult,
            )
            nc.sync.dma_start(
                out[:].rearrange("k y c -> k (y c)"), norm[:])

    nc.compile()
    return nc


_nc_cache = {}


def _get_nc(meta, st, repeat=1, single_core=False):
    key = (meta, st, repeat, single_core)
    if key not in _nc_cache:
        _nc_cache[key] = build_nc(meta, st, repeat=repeat,
                                  single_core=single_core)
    return _nc_cache[key]


def prep_in_maps(inputs):
    """Host prep: sort rows by segment; core c gets all rows of segments
    [64c, 64c+64); shared slot layout padded to the cross-core max per
    local segment; cast to fp8; partition-major. Returns (in_maps, meta,
    st)."""
    x = np.asarray(inputs["x_labels"]).astype(np.int64)
    y = np.asarray(inputs["y_labels"]).astype(np.int64)
    post = np.asarray(inputs["posterior_estimate"], dtype=np.float32)
    n = x.shape[0]
    s = (x * Y + y).astype(np.int32)
    post8 = post.astype(ml_dtypes.float8_e4m3)

    T = np.bincount(s, minlength=NSEG)
    order = np.argsort(s, kind="stable")
    s_sorted = s[order]
    seg_start = np.concatenate([[0], np.cumsum(T)])
    rk = np.arange(n) - seg_start[s_sorted]

    # local segment i spans [off[i], off[i+1]); 128-aligned after every
    # 16th segment so no boundary tile pairs PSUM row 15 with row 16.
    Lmax = T.reshape(NCORES, LSEG).max(axis=0)
    off = np.zeros(LSEG + 1, np.int64)
    cum = 0
    for i in range(LSEG):
        off[i] = cum
        cum += int(Lmax[i])
        if (i & 15) == 15:
            cum = (cum + 127) // 128 * 128
    off[LSEG] = cum
    assert cum % 128 == 0
    nt = int(cum) // 128

    lo = np.arange(nt, dtype=np.int64) * 128
    segA = np.searchsorted(off, lo, side="right") - 1
    segB = np.searchsorted(off, lo + 127, side="right") - 1
    assert np.all(segB - segA <= 1), "tile spans 3+ segments"
    q = np.where(segA == segB, 128, off[segB] - lo)
    meta = (nt, tuple(int(v) for v in segA), tuple(int(v) for v in q))

    core_of = s_sorted >> 6         # global seg // 64
    dst_all = off[s_sorted & 63] + rk
    in_maps = []
    for c in range(NCORES):
        sel = core_of == c
        pad = np.zeros((nt * P, C), ml_dtypes.float8_e4m3)
        pad[dst_all[sel]] = post8[order[sel]]
        in_maps.append({
            "post": np.ascontiguousarray(
                pad.reshape(nt, P, C).transpose(1, 0, 2).reshape(P, nt * C)),
        })
    return in_maps, meta, ST


def kernel(x_labels, y_labels, posterior_estimate, _trace=False,
           _tmpdir=None):
    global last_results
    in_maps, meta, st = prep_in_maps(dict(
        x_labels=x_labels, y_labels=y_labels,
        posterior_estimate=posterior_estimate,
    ))
    nc = _get_nc(meta, st)
    kwargs = {}
    if _trace:
        kwargs.update(trace=True, tmpdir=_tmpdir)
    res = run_bass_kernel_spmd(nc, in_maps, list(range(NCORES)), **kwargs)
    last_results = res
    return np.concatenate([res.results[c]["out"] for c in range(NCORES)],
                          axis=0)
